# revision 8
# baseline (speedup 1.0000x reference)
"""Trainium2 Bass kernel for nn_RecommendationLoss.

Reference math (B=8192, L=1024, one positive label per row at a valid index):
  mask[b,l]  = l < len[b]
  bce_per[b] = sum_l mask*bce_el / (L * len)  where bce_el = -(lab*ln(s) + (1-lab)*ln(1-s))
  bce        = mean_b bce_per
  chosen[b]  = s[b, pos_b]
  hinge[b]   = sum_l neg_mask*relu(margin + s - chosen) / (len-1)   [valid iff len>=2]
  hinge      = sum_b hinge[b] / count(len>=2)
  sim        = -mean(similarity)
  out        = (hinge + bce + sim, hinge, bce, sim)

Device computes, per row (via per-128-row tiles, 8 tiles per core, 8 cores):
  chosen = sum_l labels*s                      (DVE tensor_tensor_reduce)
  sm     = (iota < len) * s                    (GpSimd scalar_tensor_tensor)
  A      = sum_l ln(1 - sm)                    (ACT Ln with accum_out; masked-out l give ln(1)=0)
  Eraw   = sum_l relu(sm + margin - chosen)    (DVE 2-op tensor_scalar with accum_out)
  E      = Eraw - (L - len)*relu(margin - chosen)   [tail correction, per-row scalars]
  bce row sum   = -(ln(chosen) + A - ln(1-chosen))
  hinge row val = (E - margin) * [len>=2]/(len-1)
Host does the trivial 1-D pieces (sim mean, valid count, final scalar combine) in f64.
"""

import sys

for _p in ("/opt/trn_rl_repo", "/opt/trn_rl_repo/concourse"):
    if _p not in sys.path:
        sys.path.insert(0, _p)

import numpy as np

MARGIN = 0.1
B, L = 8192, 1024
N_CORES = 8
ROWS_PER_CORE = B // N_CORES      # 1024
P = 128                           # partitions
NT = ROWS_PER_CORE // P           # 8 tiles per core

_COMPILED = None


def _build():
    """Build + compile the per-core Bass program (same program on all cores)."""
    import concourse.bacc as bacc
    import concourse.tile as tile
    from concourse import mybir
    from concourse.alu_op_type import AluOpType as alu

    f32 = mybir.dt.float32
    AF = mybir.ActivationFunctionType

    nc = bacc.Bacc("TRN2", target_bir_lowering=False, debug=False,
                   num_devices=N_CORES)

    scores = nc.dram_tensor("scores", [ROWS_PER_CORE, L], f32, kind="ExternalInput").ap()
    labels = nc.dram_tensor("labels", [ROWS_PER_CORE, L], f32, kind="ExternalInput").ap()
    # per-row length-derived values, laid out [P, NT]: column t = rows of tile t
    lens_d = nc.dram_tensor("lens", [P, NT], f32, kind="ExternalInput").ap()
    rl_d = nc.dram_tensor("rl", [P, NT], f32, kind="ExternalInput").ap()      # 1/len
    hv_d = nc.dram_tensor("hv", [P, NT], f32, kind="ExternalInput").ap()      # [len>=2]/(len-1)
    tail_d = nc.dram_tensor("tail", [P, NT], f32, kind="ExternalInput").ap()  # L - len
    out_d = nc.dram_tensor("out", [P, 2], f32, kind="ExternalOutput").ap()

    with tile.TileContext(nc) as tc:
        with (
            tc.tile_pool(name="const", bufs=1) as const,
            tc.tile_pool(name="io", bufs=3) as io,
            tc.tile_pool(name="work", bufs=3) as work,
            tc.tile_pool(name="stats", bufs=1) as stats,
        ):
            iota = const.tile([P, L], f32)
            nc.gpsimd.iota(iota, pattern=[[1, L]], base=0, channel_multiplier=0,
                           allow_small_or_imprecise_dtypes=True)
            zeros = const.tile([P, L], f32)
            nc.gpsimd.memset(zeros, 0.0)
            lens_sb = const.tile([P, NT], f32)
            nc.sync.dma_start(out=lens_sb, in_=lens_d)
            rl_sb = const.tile([P, NT], f32)
            nc.sync.dma_start(out=rl_sb, in_=rl_d)
            hv_sb = const.tile([P, NT], f32)
            nc.sync.dma_start(out=hv_sb, in_=hv_d)
            tail_sb = const.tile([P, NT], f32)
            nc.sync.dma_start(out=tail_sb, in_=tail_d)

            chosen_all = stats.tile([P, NT], f32)
            mc_all = stats.tile([P, NT], f32)      # margin - chosen
            A_all = stats.tile([P, NT], f32)
            Eraw_all = stats.tile([P, NT], f32)

            for t in range(NT):
                rows = slice(t * P, (t + 1) * P)
                s_t = io.tile([P, L], f32)
                nc.sync.dma_start(out=s_t, in_=scores[rows, :])
                lab_t = io.tile([P, L], f32)
                nc.sync.dma_start(out=lab_t, in_=labels[rows, :])

                # chosen = sum_l labels * s
                junk = work.tile([P, L], f32)
                nc.vector.scalar_tensor_tensor(
                    out=junk, in0=lab_t, scalar=0.0, in1=s_t,
                    op0=alu.bypass, op1=alu.mult,
                    accum_out=chosen_all[:, t:t + 1])
                # mc = margin - chosen
                nc.vector.tensor_scalar(
                    out=mc_all[:, t:t + 1], in0=chosen_all[:, t:t + 1],
                    scalar1=-1.0, scalar2=MARGIN, op0=alu.mult, op1=alu.add)
                # mask = (iota < len)  [DVE tensor_scalar: 2x mode]
                mask = work.tile([P, L], f32)
                nc.vector.tensor_scalar(
                    out=mask, in0=iota, scalar1=lens_sb[:, t:t + 1],
                    scalar2=None, op0=alu.is_lt)
                # sm = mask * s   [gpsimd tensor_tensor]
                sm = work.tile([P, L], f32)
                nc.gpsimd.tensor_tensor(out=sm, in0=mask, in1=s_t, op=alu.mult)
                # A = sum_l ln(1 - sm)
                l1m = work.tile([P, L], f32)
                nc.scalar.activation(
                    out=l1m, in_=sm, func=AF.Ln, scale=-1.0, bias=1.0,
                    accum_out=A_all[:, t:t + 1])
                # Eraw = sum_l relu(sm + mc)  == sum_l max(sm + mc, zeros)
                q2 = work.tile([P, L], f32)
                nc.vector.scalar_tensor_tensor(
                    out=q2, in0=sm, scalar=mc_all[:, t:t + 1], in1=zeros,
                    op0=alu.add, op1=alu.max,
                    accum_out=Eraw_all[:, t:t + 1])

            # ---- final batched per-row math on [P, NT] tiles (tiny) ----
            Cc = stats.tile([P, NT], f32)
            nc.scalar.activation(out=Cc, in_=chosen_all, func=AF.Ln)
            Btl = stats.tile([P, NT], f32)
            nc.scalar.activation(out=Btl, in_=chosen_all, func=AF.Ln,
                                 scale=-1.0, bias=1.0)
            relumc = stats.tile([P, NT], f32)
            nc.vector.tensor_scalar(out=relumc, in0=mc_all, scalar1=0.0,
                                    scalar2=None, op0=alu.max)
            corr = stats.tile([P, NT], f32)
            nc.vector.tensor_mul(corr, tail_sb, relumc)
            E_all = stats.tile([P, NT], f32)
            nc.vector.tensor_sub(E_all, Eraw_all, corr)

            out_sb = stats.tile([P, 2], f32)
            # hinge partial: sum_t (E - margin) * hv
            t5 = stats.tile([P, NT], f32)
            nc.vector.tensor_scalar(out=t5, in0=E_all, scalar1=-MARGIN,
                                    scalar2=None, op0=alu.add)
            junk2 = stats.tile([P, NT], f32)
            nc.vector.scalar_tensor_tensor(
                out=junk2, in0=t5, scalar=0.0, in1=hv_sb,
                op0=alu.bypass, op1=alu.mult, accum_out=out_sb[:, 1:2])
            # bce partial: sum_t (ln(chosen) + A - ln(1-chosen)) * (1/len)
            t7 = stats.tile([P, NT], f32)
            nc.vector.tensor_sub(t7, Cc, Btl)
            t8 = stats.tile([P, NT], f32)
            nc.vector.tensor_add(t8, t7, A_all)
            junk3 = stats.tile([P, NT], f32)
            nc.vector.scalar_tensor_tensor(
                out=junk3, in0=t8, scalar=0.0, in1=rl_sb,
                op0=alu.bypass, op1=alu.mult, accum_out=out_sb[:, 0:1])

            nc.sync.dma_start(out=out_d, in_=out_sb)

    nc.compile()
    return nc


def _get_compiled():
    global _COMPILED
    if _COMPILED is None:
        _COMPILED = _build()
    return _COMPILED


def _make_in_maps(scores, labels, lens_f64):
    in_maps = []
    for c in range(N_CORES):
        rows = slice(c * ROWS_PER_CORE, (c + 1) * ROWS_PER_CORE)
        lv = lens_f64[rows].reshape(NT, P).T          # [P, NT], col t = tile t rows
        in_maps.append({
            "scores": np.ascontiguousarray(scores[rows], dtype=np.float32),
            "labels": np.ascontiguousarray(labels[rows], dtype=np.float32),
            "lens": np.ascontiguousarray(lv, dtype=np.float32),
            "rl": np.ascontiguousarray(1.0 / lv, dtype=np.float32),
            "hv": np.ascontiguousarray(
                np.where(lv >= 2.0, 1.0 / np.maximum(lv - 1.0, 1.0), 0.0),
                dtype=np.float32),
            "tail": np.ascontiguousarray(float(L) - lv, dtype=np.float32),
        })
    return in_maps


LAST_RESULTS = None  # BassKernelResults of the most recent run (for profiling)


def kernel(scores, candidate_lengths, labels, similarity_top_cand,
           _trace=False, _trace_kwargs=None):
    from concourse.bass_utils import run_bass_kernel_spmd

    global LAST_RESULTS
    nc = _get_compiled()

    scores = np.asarray(scores)
    labels = np.asarray(labels)
    lens_f64 = np.asarray(candidate_lengths).astype(np.float64)
    sim = np.asarray(similarity_top_cand).astype(np.float64)

    in_maps = _make_in_maps(scores, labels, lens_f64)
    res = run_bass_kernel_spmd(
        nc, in_maps, core_ids=list(range(N_CORES)),
        trace=_trace, **(_trace_kwargs or {}))
    LAST_RESULTS = res

    bsum = 0.0
    hsum = 0.0
    for c in range(N_CORES):
        o = np.asarray(res.results[c]["out"], dtype=np.float64)  # [P, 2]
        bsum += o[:, 0].sum()
        hsum += o[:, 1].sum()

    vcnt = float(np.count_nonzero(lens_f64 >= 2.0))
    bce = -bsum / (float(L) * float(B))
    hinge = hsum / vcnt if vcnt > 0 else 0.0
    sim_loss = -sim.mean()
    combined = hinge + bce + sim_loss
    return np.array([combined, hinge, bce, sim_loss], dtype=np.float32)
